# revision 1
# baseline (speedup 1.0000x reference)
"""AgentAttention Trainium2 kernel — data-parallel over batch on 8 NeuronCores.

Per core: 2 batch entries x 2 modalities. Host pre-transposes inputs to
channel-major bf16; device computes qkv projections, two-stage agent
attention (softmax without max-subtraction, biases folded into precomputed
exp tables, normalizations folded into tiny per-head tensors), depthwise
3x3 conv via diagonal matmuls on shifted padded access patterns, and the
output projection with bias via a K=1 ones matmul.
"""
import os
os.environ.setdefault("BY_DEFAULT_DISABLE_SUBTILE_DEPS", "1")
import numpy as np

B, N, C, HEADS, AGENT, HW = 16, 3136, 256, 8, 49, 56
DH, POOL = C // HEADS, 7
SCALE = DH ** -0.5
NCORES = 8
B_LOC = B // NCORES
NT = 448            # token tile (8 image rows)
NTILES = N // NT    # 7
NC_ = 112           # token chunk for transposes / proj
PW = HW + 2         # 58
PN = PW * PW + 2    # 3366 (2 tail cols so the last dwc window stays in bounds)
HP = 4              # head pairs


def _resize_matrix():
    R = np.zeros((HW, POOL), np.float64)
    s = POOL / HW
    for i in range(HW):
        src = (i + 0.5) * s - 0.5
        j0 = int(np.floor(src)); frac = src - j0
        for j, wgt in ((j0, 1 - frac), (j0 + 1, frac)):
            j = min(max(j, 0), POOL - 1)
            R[i, j] += wgt
    return R.astype(np.float32)


def _host_prep(inputs):
    R = _resize_matrix()
    d = {
        'wqkv': np.zeros((2, 2, 128, 768), np.float32),
        'wproj': np.zeros((2, 4, 64, C), np.float32),
        'wproj2': np.zeros((2, 2, 128, C), np.float32),
        'wdiag': np.zeros((2, 9, 2, 128, 128), np.float32),
        'projb': np.zeros((2, 1, C), np.float32),
        'exppb': np.zeros((2, HP, 98, N), np.float32),
        'expab': np.zeros((2, HP, 98, N), np.float32),
        'ident': np.eye(128, dtype=np.float32),
        's2base': np.zeros((98, 98), np.float32),
    }
    d['s2base'][0:49, 64] = 1.0
    d['s2base'][49:98, 96] = 1.0
    for mi, pre in enumerate(('rgb', 'depth')):
        g = lambda nm: np.asarray(inputs[f'{pre}_{nm}'], np.float32)
        qw = g('q_w') * SCALE
        kvw = g('kv_w')
        wall = np.concatenate([qw.T, kvw[:C].T / 64.0, kvw[C:].T], axis=1)
        d['wqkv'][mi] = wall.reshape(2, 128, 768)
        pw = g('proj_w')
        d['wproj'][mi] = pw.T.reshape(4, 64, C)
        d['wproj2'][mi] = pw.T.reshape(2, 128, C)
        dw = g('dwc_w')[:, :, 0, :]
        for t in range(9):
            dy, dx = t // 3, t % 3
            for cc in range(2):
                d['wdiag'][mi, t, cc] = np.diag(dw[dy, dx, cc * 128:(cc + 1) * 128])
        d['projb'][mi, 0] = g('proj_b') + g('dwc_b') @ pw.T
        rs = lambda t4: np.einsum('ip,hapq,jq->haij', R, t4, R).reshape(HEADS, AGENT, N)
        pb = rs(g('an_bias')) + (g('ah_bias') + g('aw_bias')).reshape(HEADS, AGENT, N)
        abT = rs(g('na_bias')) + (g('ha_bias') + g('wa_bias')).reshape(HEADS, N, AGENT).transpose(0, 2, 1)
        for name, tab in (('exppb', pb), ('expab', abT)):
            e = np.exp(tab)
            for hp in range(HP):
                d[name][mi, hp, :49] = e[2 * hp]
                d[name][mi, hp, 49:] = e[2 * hp + 1]
    return d


def _build_bass():
    import concourse.bass as bass
    import concourse.mybir as mybir
    from concourse import bacc, tile
    from contextlib import ExitStack

    BF = mybir.dt.bfloat16
    F32 = mybir.dt.float32
    A = mybir.AluOpType
    AF = mybir.ActivationFunctionType
    X = mybir.AxisListType.X

    nc = bacc.Bacc("TRN2", target_bir_lowering=False)
    xt = nc.dram_tensor('xt', [4, 2, 128, N], BF, kind="ExternalInput")
    wqkv = nc.dram_tensor('wqkv', [2, 2, 128, 768], BF, kind="ExternalInput")
    wproj = nc.dram_tensor('wproj', [2, 4, 64, C], BF, kind="ExternalInput")
    wproj2 = nc.dram_tensor('wproj2', [2, 2, 128, C], BF, kind="ExternalInput")
    wdiag = nc.dram_tensor('wdiag', [2, 9, 2, 128, 128], BF, kind="ExternalInput")
    projb = nc.dram_tensor('projb', [2, 1, C], BF, kind="ExternalInput")
    exppb = nc.dram_tensor('exppb', [2, HP, 98, N], BF, kind="ExternalInput")
    expab = nc.dram_tensor('expab', [2, HP, 98, N], BF, kind="ExternalInput")
    ident = nc.dram_tensor('ident', [128, 128], BF, kind="ExternalInput")
    s2base = nc.dram_tensor('s2base', [98, 98], BF, kind="ExternalInput")
    out = nc.dram_tensor('out', [4, N, C], F32, kind="ExternalOutput")

    with tile.TileContext(nc) as tc, ExitStack() as ctx:
        const = ctx.enter_context(tc.tile_pool(name="const", bufs=1))
        feats = ctx.enter_context(tc.tile_pool(name="feats", bufs=1))
        work = ctx.enter_context(tc.tile_pool(name="work", bufs=3))
        tiny = ctx.enter_context(tc.tile_pool(name="tiny", bufs=1))
        psQ = ctx.enter_context(tc.tile_pool(name="psQ", bufs=2, space="PSUM"))
        psW = ctx.enter_context(tc.tile_pool(name="psW", bufs=2, space="PSUM"))
        psUV = ctx.enter_context(tc.tile_pool(name="psUV", bufs=1, space="PSUM"))

        # ---- constants ----
        idt = const.tile([128, 128], BF, tag="ident", name="ident")
        nc.sync.dma_start(out=idt[:, :], in_=ident[:, :])
        wq_s, wp_s, wd_s, pbrow = {}, {}, {}, {}
        for m in range(2):
            for kc in range(2):
                t_ = const.tile([128, 768], BF, tag=f"wqkv{m}{kc}", name=f"wqkv{m}{kc}")
                nc.sync.dma_start(out=t_[:, :], in_=wqkv[m, kc])
                wq_s[(m, kc)] = t_
            for kc in range(4):
                t_ = const.tile([64, C], BF, tag=f"wproj{m}{kc}", name=f"wproj{m}{kc}")
                nc.sync.dma_start(out=t_[:, :], in_=wproj[m, kc])
                wp_s[(m, kc)] = t_
            for kc in range(2):
                t_ = const.tile([128, C], BF, tag=f"wproj2_{m}{kc}", name=f"wproj2_{m}{kc}")
                nc.sync.dma_start(out=t_[:, :], in_=wproj2[m, kc])
                wp_s[(m, 'd', kc)] = t_
            for t in range(9):
                for cc in range(2):
                    t_ = const.tile([128, 128], BF, tag=f"wdiag{m}{t}{cc}", name=f"wdiag{m}{t}{cc}")
                    nc.sync.dma_start(out=t_[:, :], in_=wdiag[m, t, cc])
                    wd_s[(m, t, cc)] = t_
            t_ = const.tile([1, C], BF, tag=f"projb{m}", name=f"projb{m}")
            nc.sync.dma_start(out=t_[:, :], in_=projb[m])
            pbrow[m] = t_
        onesr = const.tile([1, NC_], BF, tag="ones", name="ones")
        nc.vector.memset(onesr[:, :], 1.0)

        for b in range(B_LOC):
            # ---------------- phase A: qkv for both modalities ----------------
            qT, kT, vT, vpad, pool_out = {}, {}, {}, {}, {}
            for m in range(2):
                mb = m * 2 + b
                x_s = []
                for kc in range(2):
                    t_ = feats.tile([128, N], BF, tag=f"xT{kc}", name=f"xT{kc}")
                    nc.sync.dma_start(out=t_[:, :], in_=xt[mb, kc])
                    x_s.append(t_)
                qT[m] = [feats.tile([128, N], BF, tag=f"qT{m}{c}", name=f"qT{m}{c}") for c in range(2)]
                kT[m] = [feats.tile([128, N], BF, tag=f"kT{m}{c}", name=f"kT{m}{c}") for c in range(2)]
                vT[m] = [feats.tile([128, N], BF, tag=f"vT{m}{c}", name=f"vT{m}{c}") for c in range(2)]
                vpad[m] = [feats.tile([128, PN], BF, tag=f"vpad{m}{c}", name=f"vpad{m}{c}") for c in range(2)]
                for cc in range(2):
                    vp = vpad[m][cc]
                    nc.vector.memset(vp[:, 0:PW], 0.0)                # top pad row
                    nc.vector.memset(vp[:, PN - PW - 2:PN], 0.0)      # bottom pad row + tail
                    sides = vp[:, 0:PW * PW].rearrange("p (r c) -> p r c", c=PW)[:, 1:57, 0:1]
                    nc.vector.memset(sides, 0.0)
                    sides2 = vp[:, 0:PW * PW].rearrange("p (r c) -> p r c", c=PW)[:, 1:57, 57:58]
                    nc.vector.memset(sides2, 0.0)
                for t in range(NTILES):
                    sl = bass.ts(t, NT)
                    for mo in range(6):
                        ps = psQ.tile([128, NT], F32, tag="qkv", name="qkv")
                        for kc in range(2):
                            nc.tensor.matmul(ps[:, :], wq_s[(m, kc)][:, mo * 128:(mo + 1) * 128],
                                             x_s[kc][:, sl], start=(kc == 0), stop=(kc == 1))
                        cc = mo % 2
                        if mo < 2:
                            nc.scalar.activation(qT[m][cc][:, sl], ps[:, :], AF.Copy)
                        elif mo < 4:
                            nc.scalar.activation(kT[m][cc][:, sl], ps[:, :], AF.Copy)
                        else:
                            nc.vector.tensor_copy(vT[m][cc][:, sl], ps[:, :])
                # fill padded image copies (row-structured SBUF->SBUF DMA)
                for cc in range(2):
                    vpv = vpad[m][cc][:, 0:PW * PW].rearrange("p (r c) -> p r c", c=PW)
                    nc.sync.dma_start(out=vpv[:, 1:57, 1:57],
                                      in_=vT[m][cc][:, :].rearrange("p (r c) -> p r c", c=HW))
                # agent pooling: strided 2-pass sum over qT chunks
                for cc in range(2):
                    tmp = work.tile([128, 392], F32, tag="pooltmp", name="pooltmp")
                    src = qT[m][cc][:, :].rearrange("p (g j) -> p g j", j=8)
                    nc.vector.tensor_reduce(tmp[:, :], src, op=A.add, axis=X)
                    po = tiny.tile([128, 49], F32, tag=f"pool{m}{cc}", name=f"pool{m}{cc}{b}")
                    src2 = tmp[:, :].rearrange("p (wr rr wc) -> p wr wc rr", wr=7, rr=8)
                    nc.vector.tensor_reduce(po[:, :], src2, op=A.add, axis=X)
                    pool_out[(m, cc)] = po

            # block-diag stationaries (agents from the OTHER modality).
            # Stored at the same partition offset as the kT/qT slice they
            # pair with (matmul requires equal base partitions).
            lhs1, lhs2 = {}, {}
            for m in range(2):
                other = 1 - m
                for hp in range(HP):
                    cc, r0 = divmod(hp, 2)
                    p0 = r0 * 64
                    t1 = tiny.tile([128, 98], BF, tag=f"lhs1_{m}{hp}", name=f"lhs1_{m}{hp}{b}")
                    nc.vector.memset(t1[p0:p0 + 64, :], 0.0)
                    src = pool_out[(other, cc)]
                    nc.gpsimd.dma_start(out=t1[p0:p0 + 32, 0:49], in_=src[p0:p0 + 32, :])
                    nc.gpsimd.dma_start(out=t1[p0 + 32:p0 + 64, 49:98], in_=src[p0 + 32:p0 + 64, :])
                    lhs1[(m, hp)] = t1[p0:p0 + 64, :]
                    t2 = tiny.tile([128, 98], BF, tag=f"lhs2_{m}{hp}", name=f"lhs2_{m}{hp}{b}")
                    nc.vector.tensor_scalar_mul(t2[p0:p0 + 64, :], t1[p0:p0 + 64, :], 1.0 / (64.0 * SCALE))
                    lhs2[(m, hp)] = t2[p0:p0 + 64, :]

            # ---------------- phase B: stage 1 ----------------
            lhsS2 = {}
            for m in range(2):
                uvps = [psUV.tile([128, 448], F32, tag=f"acc{g}", name=f"uv{g}") for g in range(2)]
                z1p = [tiny.tile([98, NTILES], F32, tag=f"z1p{m}{hp}", name=f"z1p{m}{hp}{b}") for hp in range(HP)]
                for t in range(NTILES):
                    sl = bass.ts(t, NT)
                    p1 = []
                    for hp in range(HP):
                        cc, r0 = divmod(hp, 2)
                        ps = psW.tile([98, NT], F32, tag="tmp", name="tmp")
                        nc.tensor.matmul(ps[:, :], lhs1[(m, hp)],
                                         kT[m][cc][r0 * 64:(r0 + 1) * 64, sl],
                                         start=True, stop=True)
                        pbs = work.tile([98, NT], BF, tag="pbs", name="pbs")
                        nc.sync.dma_start(out=pbs[:, :], in_=exppb[m, hp, :, sl])
                        pe = work.tile([98, NT], BF, tag=f"p1_{hp}", name=f"p1_{hp}", bufs=2)
                        nc.scalar.activation(pe[:, :], ps[:, :], AF.Exp)
                        nc.vector.scalar_tensor_tensor(
                            pe[:, :], pe[:, :], 1.0, pbs[:, :],
                            op0=A.mult, op1=A.mult, accum_out=z1p[hp][:, t:t + 1])
                        p1.append(pe)
                    for q in range(4):
                        qs = slice(q * NC_, (q + 1) * NC_)
                        p1t = work.tile([112, 392], BF, tag="p1t", name="p1t")
                        for hp in range(HP):
                            pst = psW.tile([112, 98], BF, tag="tmp", name="tmp")
                            nc.tensor.transpose(pst[:, :], p1[hp][:, qs], idt[0:98, 0:98])
                            if hp % 2 == 0:
                                nc.scalar.activation(p1t[:, hp * 98:(hp + 1) * 98], pst[:, :], AF.Copy)
                            else:
                                nc.vector.tensor_copy(p1t[:, hp * 98:(hp + 1) * 98], pst[:, :])
                        vt = work.tile([112, 256], BF, tag="vtm", name="vtm")
                        for cc in range(2):
                            pst = psW.tile([112, 128], BF, tag="tmp", name="tmp")
                            nc.tensor.transpose(pst[:, :],
                                                vT[m][cc][:, t * NT + q * NC_:t * NT + (q + 1) * NC_],
                                                idt[:, :])
                            nc.vector.tensor_copy(vt[:, cc * 128:(cc + 1) * 128], pst[:, :])
                        for g in range(2):
                            nc.tensor.matmul(uvps[g][:, 0:196],
                                             vt[:, g * 128:(g + 1) * 128],
                                             p1t[:, g * 196:(g + 1) * 196],
                                             start=(t == 0 and q == 0),
                                             stop=(t == NTILES - 1 and q == 3))
                # finalize: stage-2 stationary [98, 97] per hp
                # cols 0-63 = UV' blockdiag, col 64 = ones(even head rows),
                # col 96 = ones(odd head rows) -> Z2 lands at psum rows 64/96
                for hp in range(HP):
                    g, hp2 = divmod(hp, 2)
                    z1 = tiny.tile([98, 1], F32, tag=f"z1{m}{hp}", name=f"z1{m}{hp}{b}")
                    nc.vector.tensor_reduce(z1[:, :], z1p[hp][:, :], op=A.add, axis=X)
                    z1i = tiny.tile([98, 1], F32, tag=f"z1i{m}{hp}", name=f"z1i{m}{hp}{b}")
                    nc.vector.reciprocal(z1i[:, :], z1[:, :])
                    s2 = tiny.tile([98, 97], BF, tag=f"lhsS2_{m}{hp}", name=f"lhsS2_{m}{hp}{b}")
                    nc.sync.dma_start(out=s2[:, :], in_=s2base[:, 0:97])
                    for h2 in range(2):
                        uvs = tiny.tile([32, 49], BF, tag=f"uvs{m}{hp}{h2}", name=f"uvs{m}{hp}{h2}{b}")
                        nc.vector.tensor_copy(
                            uvs[:, :],
                            uvps[g][hp2 * 64 + h2 * 32:hp2 * 64 + (h2 + 1) * 32,
                                    hp2 * 98 + h2 * 49:hp2 * 98 + (h2 + 1) * 49])
                        pst = psW.tile([49, 32], BF, tag="tmp", name="tmp")
                        nc.tensor.transpose(pst[:, :], uvs[:, :], idt[0:32, 0:32])
                        uvt_s = tiny.tile([49, 32], BF, tag=f"uvt{m}{hp}{h2}", name=f"uvt{m}{hp}{h2}{b}")
                        nc.scalar.activation(uvt_s[:, :], pst[:, :], AF.Copy)
                        nc.gpsimd.dma_start(out=s2[h2 * 49:(h2 + 1) * 49, h2 * 32:(h2 + 1) * 32],
                                            in_=uvt_s[:, :])
                    nc.vector.tensor_scalar_mul(s2[:, 0:64], s2[:, 0:64], z1i[:, 0:1])
                    lhsS2[(m, hp)] = s2

            # ---------------- phase C: stage 2 + dwc + proj ----------------
            for m in range(2):
                mb = m * 2 + b
                for t in range(NTILES):
                    sl = bass.ts(t, NT)
                    attn = []
                    for hp in range(HP):
                        cc, r0 = divmod(hp, 2)
                        ps = psW.tile([98, NT], F32, tag="tmp", name="tmp")
                        nc.tensor.matmul(ps[:, :], lhs2[(m, hp)],
                                         qT[m][cc][r0 * 64:(r0 + 1) * 64, sl],
                                         start=True, stop=True)
                        abs_ = work.tile([98, NT], BF, tag="abs", name="abs")
                        nc.sync.dma_start(out=abs_[:, :], in_=expab[m, hp, :, sl])
                        p2 = work.tile([98, NT], BF, tag="p2", name="p2")
                        nc.scalar.activation(p2[:, :], ps[:, :], AF.Exp)
                        nc.vector.tensor_tensor(p2[:, :], p2[:, :], abs_[:, :], op=A.mult)
                        pv = psW.tile([97, NT], F32, tag="tmp", name="tmp")
                        nc.tensor.matmul(pv[:, :], lhsS2[(m, hp)][:, :], p2[:, :],
                                         start=True, stop=True)
                        z2a = work.tile([1, NT], F32, tag="z2a", name="z2a")
                        z2c = work.tile([1, NT], F32, tag="z2c", name="z2c")
                        nc.vector.reciprocal(z2a[:, :], pv[64:65, :])
                        nc.vector.reciprocal(z2c[:, :], pv[96:97, :])
                        zb0 = work.tile([32, NT], F32, tag="zb0", name="zb0", bufs=2)
                        zb1 = work.tile([32, NT], F32, tag="zb1", name="zb1", bufs=2)
                        nc.gpsimd.partition_broadcast(zb0[:, :], z2a[:, :])
                        nc.gpsimd.partition_broadcast(zb1[:, :], z2c[:, :])
                        at = work.tile([64, NT], BF, tag=f"attn{hp}", name=f"attn{hp}", bufs=2)
                        nc.vector.tensor_tensor(at[0:32, :], pv[0:32, :], zb0[:, :], op=A.mult)
                        nc.vector.tensor_tensor(at[32:64, :], pv[32:64, :], zb1[:, :], op=A.mult)
                        attn.append(at)
                    dwc = []
                    for cc in range(2):
                        pd = psUV.tile([128, 464], F32, tag=f"acc{cc}", name=f"dwc{cc}")
                        for tap in range(9):
                            dy, dx = tap // 3, tap % 3
                            base = (t * 8 + dy) * PW + dx
                            nc.tensor.matmul(pd[:, :], wd_s[(m, tap, cc)][:, :],
                                             vpad[m][cc][:, base:base + 464],
                                             start=(tap == 0), stop=(tap == 8))
                        dd = work.tile([128, NT], BF, tag=f"dwcs{cc}", name=f"dwcs{cc}")
                        nc.scalar.activation(
                            dd[:, :].rearrange("p (r c) -> p r c", c=HW),
                            pd[:, :].rearrange("p (r c) -> p r c", c=PW)[:, :, 0:56],
                            AF.Copy)
                        dwc.append(dd)
                    for q in range(4):
                        qs = slice(q * NC_, (q + 1) * NC_)
                        pp = psQ.tile([112, C], F32, tag="proj", name="proj")
                        for hp in range(HP):
                            nc.tensor.matmul(pp[:, :], attn[hp][:, qs], wp_s[(m, hp)][:, :],
                                             start=(hp == 0), stop=False)
                        for cc in range(2):
                            nc.tensor.matmul(pp[:, :], dwc[cc][:, qs],
                                             wp_s[(m, 'd', cc)][:, :], start=False, stop=False)
                        nc.tensor.matmul(pp[:, :], onesr[:, :], pbrow[m][:, :],
                                         start=False, stop=True)
                        os_ = work.tile([112, C], F32, tag="outs", name="outs")
                        nc.scalar.activation(os_[:, :], pp[:, :], AF.Copy)
                        nc.sync.dma_start(out=out[mb, t * NT + q * NC_:t * NT + (q + 1) * NC_, :],
                                          in_=os_[:, :])
    nc.compile()
    return nc


def kernel(**inputs):
    import ml_dtypes
    from concourse.bass_utils import run_bass_kernel_spmd
    bf16 = ml_dtypes.bfloat16
    x = np.asarray(inputs['x'], np.float32)
    y = np.asarray(inputs['y'], np.float32)
    shared = {k: v.astype(bf16) for k, v in _host_prep(inputs).items()}
    nc = _build_bass()
    in_maps = []
    for core in range(NCORES):
        b0 = core * B_LOC
        xtc = np.zeros((4, 2, 128, N), bf16)
        for b in range(B_LOC):
            for mi, t in enumerate((x, y)):
                xtc[mi * 2 + b] = t[b0 + b].T.astype(bf16).reshape(2, 128, N)
        im = dict(shared)
        im['xt'] = xtc
        in_maps.append(im)
    res = run_bass_kernel_spmd(nc, in_maps, list(range(NCORES)))
    global LAST_RES
    LAST_RES = res
    xo = np.zeros((B, N, C), np.float32)
    yo = np.zeros((B, N, C), np.float32)
    for core in range(NCORES):
        o = np.asarray(res.results[core]['out'], np.float32)
        b0 = core * B_LOC
        for b in range(B_LOC):
            xo[b0 + b] = o[b]
            yo[b0 + b] = o[2 + b]
    return np.stack([xo, yo])

